# revision 55
# baseline (speedup 1.0000x reference)
"""Distributed Trainium2 (8 NeuronCores) kernel for nn_AdaptiveAttention.

Reference computation (b=2, n=2048, d=1024, 16 heads x 64):
    qkv = x @ W_qkv; q,k,v = split(qkv)
    attn = softmax(mask(q k^T / sqrt(dh)))
    out  = (attn @ v) @ W_out + b_out

Sharding: core c in [0,8) handles batch b = c//4 and head group g = c%4
(heads 4g..4g+3).  Data parallel over b, tensor parallel over heads.

Schedule (per core): a short pre-round projects q/k for the first head
pair, then FOUR fully-paired attention rounds cover the 8 (head,
i-half) combos; in every round the two concurrent heads' score matmuls
land on complementary PE row-tiles (0,0)/(64,0) since head_dim=64.
Remaining projections (v just-in-time, second-pair q/k) are woven into
rounds 0-1 using the shared score-PSUM pool, so both attn@v
accumulator pools coexist with it in exactly 8 PSUM banks.  Rounds are
paced by the Act engine (exp) at ~2.7us per j-chunk; PE rides along
with ~35% slack that absorbs the woven projections.

Each head-round's tail evacuates raw attention + softmax sums (ones
column appended to v) and fires a small per-(head, i-half) 8-rank
AllToAll sending each peer only the 256 i-columns it owns.  Chunks are
consumed a round later; normalization (reciprocal of gathered sums,
broadcast via a tiny selector matmul) and the output projection of
i-half 0 overlap round 3 and the final exchange's rank-skew window, so
the kernel tail holds only the last chunk's exchange plus i-half 1's
projection.  The mask ships as fp8 (exact for 0/1) and is cast to bf16
by the gpsimd software-DGE on the way into SBUF, halving its HBM
traffic.

Numerics: all matmuls bf16 operands with fp32 PSUM accumulation.
Softmax runs without max-subtraction (scores are O(1) by construction)
as exp(s) * mask.
"""

import numpy as np
import ml_dtypes

import concourse.bass as bass
import concourse.tile as tile
from concourse import bacc, mybir
from concourse import bass_utils

BF16 = ml_dtypes.bfloat16

B = 2
N = 2048
D = 1024
HEADS = 16
HD = 64  # head dim
SCALE = HD ** -0.5
N_CORES = 8
HPC = 4  # heads per core
IB = 1024  # i-block size (one ib2 half)
NJ = N // 128  # 16 j-chunks

_cached_nc = None
_last_in_maps = None
_last_res = None


def _build():
    nc = bacc.Bacc("TRN2", target_bir_lowering=False, debug=False,
                   num_devices=N_CORES)

    f32 = mybir.dt.float32
    bf = mybir.dt.bfloat16
    fp8 = mybir.dt.float8e4

    xt = nc.dram_tensor("xt", [D, N], bf, kind="ExternalInput")
    wqkv = nc.dram_tensor("wqkv", [D, 768], bf, kind="ExternalInput")
    # mask is exactly 0/1 so it ships as fp8 (half the HBM traffic) and
    # the gpsimd software-DGE casts it to bf16 on the way into SBUF
    maskt = nc.dram_tensor("maskt", [N, N], fp8, kind="ExternalInput")
    wout = nc.dram_tensor("wout", [D, D], bf, kind="ExternalInput")
    e8 = nc.dram_tensor("e8", [8, 512], bf, kind="ExternalInput")
    out = nc.dram_tensor("out", [N // 4, D], bf, kind="ExternalOutput")

    with tile.TileContext(nc) as tc:
        with (
            tc.tile_pool(name="res", bufs=1) as res,
            tc.tile_pool(name="dram", bufs=1, space="DRAM") as dram,
            tc.tile_pool(name="pe", bufs=4) as p_e,
            tc.tile_pool(name="pp", bufs=4) as p_p,
            tc.tile_pool(name="pao", bufs=2) as pao,
            tc.tile_pool(name="ph2", bufs=1) as p2,
            tc.tile_pool(name="ost", bufs=2) as po,
            tc.tile_pool(name="pps", bufs=2, space="PSUM") as pp_s,
            tc.tile_pool(name="ppa1", bufs=1, space="PSUM") as pp_a1,
            tc.tile_pool(name="ppa2", bufs=1, space="PSUM") as pp_a2,
        ):
            # resident tensors
            # qkt: [qT01 | qT23 | kT01 | kT23], each [128, 2048] bf16
            qkt = res.tile([128, 4 * N], bf)
            # v_aug: per j-chunk jc block of 260 cols: 4x(64 v cols + ones)
            v_aug = res.tile([128, NJ * 260], bf)
            # mask, one tile per j-chunk for fine-grained load deps
            mts = [res.tile([128, N], bf, name=f"mt{jc}") for jc in range(NJ)]
            wout_sb = res.tile([128, 8 * D], bf)
            e8_sb = res.tile([8, 512], bf)
            z65 = res.tile([128, 65], bf)  # zero lhsT for warm-keeper mms

            # AllToAll bounce buffers, one per (pair, i-half) chunk =
            # one per round, so each round boundary fires exactly one
            # collective.  Shard j = rows [130j, 130j+130) goes to rank
            # j: per head hh of the pair, rows 65hh..65hh+65 hold its
            # attention [64 rows] plus its softmax sums row; columns are
            # the 256 i-cols that rank j owns within this i-half.
            a2a_ins = [[dram.tile([8 * 130, 256], bf, name=f"a2a_in{p}{h}")
                        for h in range(2)] for p in range(2)]
            a2a_outs = [[dram.tile([8 * 130, 256], bf,
                                   name=f"a2a_out{p}{h}")
                         for h in range(2)] for p in range(2)]

            # gathered attention [pair][g] / normalized copies / sums
            at_sb = [[[p2.tile([128, 256], bf, name=f"at{h}_{p}_{g}")
                       for g in range(4)] for p in range(2)]
                     for h in range(2)]
            at_n = [[[p2.tile([128, 256], bf, name=f"an{h}_{p}_{g}")
                      for g in range(4)] for p in range(2)]
                    for h in range(2)]
            sums_sb = [[p2.tile([8, 256], bf, name=f"sm{h}_{p}")
                        for p in range(2)] for h in range(2)]
            recs = [[p2.tile([8, 256], bf, name=f"rc{h}_{p}")
                     for p in range(2)] for h in range(2)]

            nc.vector.memset(z65[:], 0.0)
            nc.vector.memset(v_aug[:], 1.0)

            # tiny warm-up AllToAll: absorbs the first-collective channel
            # setup cost during the load phase AND aligns all ranks --
            # its all-ones output is copied (value-preserving) over a
            # v_aug ones column, so every rank's round 0 attn@v gates on
            # every other rank having started.  Without this the launch
            # skew (~23us) surfaces as an entry wait on every exchange.
            cwu_in = dram.tile([8, 16], bf, name="cwu_in")
            cwu_out = dram.tile([8, 16], bf, name="cwu_out")
            wu_sb = res.tile([8, 16], bf)
            wu2_sb = res.tile([8, 16], bf)
            nc.vector.memset(wu_sb[:], 1.0)
            nc.sync.dma_start(cwu_in[:, :], wu_sb[:])
            nc.gpsimd.collective_compute(
                "AllToAll", mybir.AluOpType.bypass,
                replica_groups=[[0, 1, 2, 3, 4, 5, 6, 7]],
                ins=[cwu_in[:].opt()], outs=[cwu_out[:].opt()],
            )
            nc.sync.dma_start(wu2_sb[:], cwu_out[:, :])
            nc.vector.tensor_copy(v_aug[0:8, 64:65], wu2_sb[:, 0:1])

            pid = nc.sync.partition_id()
            goff = (pid // 4) * 520  # my batch group's a2a row base

            with (
                tc.tile_pool(name="ph0", bufs=1) as p0,
            ):
                xtr = [p0.tile([128, N], bf, name=f"xtr{k}")
                       for k in range(8)]
                wr = [p0.tile([128, 768], bf, name=f"wr{k}")
                      for k in range(8)]
                # spread bulk loads over the three DMA-capable queues
                # (sync + scalar HWDGE, gpsimd SWDGE); mask/wout queue
                # behind x/w in FIFO order, giving x/w priority
                for k in range(8):
                    (nc.sync if k % 2 == 0 else nc.scalar).dma_start(
                        xtr[k][:], xt[128 * k:128 * (k + 1), :])
                    nc.gpsimd.dma_start(wr[k][:],
                                        wqkv[128 * k:128 * (k + 1), :])
                for jc in range(NJ):
                    nc.gpsimd.dma_start(
                        mts[jc][:], maskt[128 * jc:128 * (jc + 1), :])
                for k in range(8):
                    (nc.sync if k % 2 == 0 else nc.scalar).dma_start(
                        wout_sb[:, D * k:D * (k + 1)],
                        wout[128 * k:128 * (k + 1), :])
                nc.gpsimd.dma_start(e8_sb[:], e8[:, :])

                def proj_qk_group(t_i, nb):
                    wcol = 128 * t_i
                    ps = pp_s.tile([128, 512], f32, name="ps_qk", tag="mm")
                    for k in range(8):
                        nc.tensor.matmul(
                            ps[:],
                            wr[k][:, wcol:wcol + 128],
                            xtr[k][:, 512 * nb:512 * nb + 512],
                            start=(k == 0), stop=(k == 7),
                        )
                    nc.vector.tensor_copy(
                        qkt[:, N * t_i + 512 * nb:N * t_i + 512 * nb + 512],
                        ps[:])

                def proj_v_group(jc):
                    ps = pp_s.tile([128, 256], f32, name="ps_v", tag="mm")
                    for k in range(8):
                        nc.tensor.matmul(
                            ps[:],
                            xtr[k][:, 128 * jc:128 * jc + 128],
                            wr[k][:, 512:768],
                            start=(k == 0), stop=(k == 7),
                        )
                    for h in range(4):
                        nc.vector.tensor_copy(
                            v_aug[:, 260 * jc + 65 * h:260 * jc + 65 * h + 64],
                            ps[:, 64 * h:64 * h + 64])

                # hl = head-local index (0..3) = 2*pair + hh
                def sc_iter(hl, ib2, jc):
                    pair, hh = hl // 2, hl % 2
                    q_off = N * pair
                    k_off = N * (2 + pair)
                    s_ps = pp_s.tile([128, IB], f32, name="s_ps", tag="mm")
                    for ih in range(2):
                        nc.tensor.matmul(
                            s_ps[:, 512 * ih:512 * ih + 512],
                            qkt[64 * hh:64 * hh + 64,
                                k_off + 128 * jc:k_off + 128 * jc + 128],
                            qkt[64 * hh:64 * hh + 64,
                                q_off + IB * ib2 + 512 * ih:
                                q_off + IB * ib2 + 512 * ih + 512],
                            start=True, stop=True,
                        )
                    return s_ps

                def ep_iter(ib2, jc, s_ps, eng=None):
                    # the mask multiply alternates between DVE and the
                    # otherwise-idle gpsimd engine (all-SBUF operands) so
                    # DVE keeps up with PSUM evacuation casts
                    e_t = p_e.tile([128, IB], bf, name="e_t", tag="e_t")
                    nc.scalar.activation(
                        e_t[:], s_ps[:], mybir.ActivationFunctionType.Exp)
                    p_t = p_p.tile([128, IB], bf, name="p_t", tag="p_t")
                    (eng or nc.vector).tensor_mul(
                        p_t[:], e_t[:], mts[jc][:, IB * ib2:IB * ib2 + IB])
                    return p_t

                def av_iter(hl, jc, acc, p_t):
                    for ih in range(2):
                        nc.tensor.matmul(
                            acc[:, 512 * ih:512 * ih + 512],
                            v_aug[:, 260 * jc + 65 * hl:
                                  260 * jc + 65 * hl + 65],
                            p_t[:, 512 * ih:512 * ih + 512],
                            start=(jc == 0), stop=(jc == NJ - 1),
                        )

                def warm_mm(acc):
                    nc.tensor.matmul(
                        acc[:, 0:256], z65[:], v_aug[:, 0:256],
                        start=False, stop=False,
                        skip_group_check=True,
                    )

                def round_tail(pair, ib2, accA, accB):
                    """Evacuate both heads' raw attention + sums rows,
                    scatter into the AllToAll input shards (one [65, 256]
                    block per head per target rank), and fire this
                    round's (pair, i-half) exchange."""
                    for hh, acc in ((0, accA), (1, accB)):
                        ao = pao.tile([65, IB], bf, name="ao", tag="ao")
                        nc.vector.tensor_copy(ao[:], acc[:])
                        for j in range(8):
                            nc.sync.dma_start(
                                a2a_ins[pair][ib2][
                                    130 * j + 65 * hh:
                                    130 * j + 65 * hh + 65, :],
                                ao[:, 256 * (j % 4):256 * (j % 4) + 256])
                    nc.gpsimd.collective_compute(
                        "AllToAll",
                        mybir.AluOpType.bypass,
                        replica_groups=[[0, 1, 2, 3, 4, 5, 6, 7]],
                        ins=[a2a_ins[pair][ib2][:].opt()],
                        outs=[a2a_outs[pair][ib2][:].opt()],
                    )

                def emit_chunk_reads(pair, ib2, ms):
                    # per source rank g: attention blocks [64, 256] at
                    # rows 130g (hh0) and 130g+65 (hh1); sums rows sit at
                    # 130g + 65hh + 64 = every 65th row.  Sync-queue DMAs
                    # (whose collective-completion waits are reliably
                    # enforced), emitted a full round after the chunk
                    # fires so the wait is short; the wait_until stamp
                    # keeps the scheduler from hoisting them earlier.
                    src = a2a_outs[pair][ib2]
                    with tc.tile_wait_until(ms):
                        for g in range(4):
                            for hh in range(2):
                                nc.sync.dma_start(
                                    at_sb[ib2][pair][g][
                                        64 * hh:64 * hh + 64, :],
                                    src[bass.ds(goff + 130 * g + 65 * hh,
                                                64), :])
                        nc.sync.dma_start(
                            sums_sb[ib2][pair][:],
                            src[bass.ds(goff + 64, 8, 65), :])

                def emit_norm(pair, ib2, ms):
                    with tc.tile_wait_until(ms):
                        with nc.allow_low_precision(
                                reason="softmax recip bf16"):
                            nc.vector.reciprocal(recs[ib2][pair][:],
                                                 sums_sb[ib2][pair][:])

                def emit_bcmul(pair, ib2, g, ms):
                    with tc.tile_wait_until(ms):
                        bc = pp_s.tile([128, 256], f32, name="bc", tag="mm")
                        nc.tensor.matmul(bc[:],
                                         e8_sb[:, 128 * g:128 * g + 128],
                                         recs[ib2][pair][:], start=True,
                                         stop=True)
                        nc.vector.tensor_mul(at_n[ib2][pair][g][:],
                                             at_sb[ib2][pair][g][:], bc[:])

                def outproj_group(ib2, io, nh, ms):
                    with tc.tile_wait_until(ms):
                        ps = pp_s.tile([128, 512], f32, name="ps_o",
                                       tag="mm")
                        for ki, (p, g) in enumerate(
                                [(p, g) for g in range(4)
                                 for p in range(2)]):
                            nc.tensor.matmul(
                                ps[:],
                                at_n[ib2][p][g][:, 128 * io:128 * io + 128],
                                wout_sb[:, D * (2 * g + p) + 512 * nh:
                                        D * (2 * g + p) + 512 * nh + 512],
                                start=(ki == 0), stop=(ki == 7),
                            )
                        ot = po.tile([128, 512], bf, name="ot", tag="ot")
                        nc.vector.tensor_copy(ot[:], ps[:])
                        nc.sync.dma_start(
                            out[256 * ib2 + 128 * io:
                                256 * ib2 + 128 * io + 128,
                                512 * nh:512 * nh + 512],
                            ot[:])

                def new_acc(pool):
                    return pool.tile([65, IB], f32, name="acc", tag="acc")

                # pre-round: only what round 0 jc0 needs (q i-cols
                # 0-1024, first k chunk, first v chunk); the rest of
                # qT01/kT01 weaves into round 0 ahead of its first use
                proj_qk_group(0, 0)
                proj_qk_group(0, 1)
                proj_qk_group(2, 0)
                proj_v_group(0)

                # ---- four fully-paired rounds; weave lists give each
                # round's extra PE work as (jc -> thunk) slots
                def run_round(pair, ib2, weave, reads):
                    hlA, hlB = 2 * pair, 2 * pair + 1
                    accA = new_acc(pp_a1)
                    accB = new_acc(pp_a2)
                    pA = ep_iter(ib2, 0, sc_iter(hlA, ib2, 0))
                    pB = ep_iter(ib2, 0, sc_iter(hlB, ib2, 0), nc.gpsimd)
                    for jc in range(NJ):
                        if jc + 1 < NJ:
                            pA_n = ep_iter(ib2, jc + 1,
                                           sc_iter(hlA, ib2, jc + 1))
                            pB_n = ep_iter(ib2, jc + 1,
                                           sc_iter(hlB, ib2, jc + 1),
                                           nc.gpsimd)
                        av_iter(hlA, jc, accA, pA)
                        av_iter(hlB, jc, accB, pB)
                        for job in weave.get(jc, ()):
                            job()
                        if jc not in (0, NJ - 1):
                            warm_mm(accA)
                        pA, pB = pA_n, pB_n
                    round_tail(pair, ib2, accA, accB)
                    for rd in reads:
                        emit_chunk_reads(*rd)

                # R0: pair 0, i-half 0; weave v just-in-time plus
                # the rest of kT01 (nb_k first read at jc 4k) and the
                # qT01 half that round 1 needs
                w0 = {jc: [lambda jc=jc: proj_v_group(jc + 1)]
                      for jc in range(NJ - 1)}
                w0[1] = w0[1] + [lambda: proj_qk_group(2, 1)]
                w0[4] = w0[4] + [lambda: proj_qk_group(2, 2)]
                w0[8] = w0[8] + [lambda: proj_qk_group(2, 3)]
                w0[11] = w0[11] + [lambda: proj_qk_group(0, 2)]
                w0[13] = w0[13] + [lambda: proj_qk_group(0, 3)]
                run_round(0, 0, w0, [])

                # R1: pair 0, i-half 1; weave all pair-1 projections
                # (first needed by R2)
                w1 = {2 * i + 1: [lambda t=t, nb=nb: proj_qk_group(t, nb)]
                      for i, (t, nb) in enumerate(
                          [(1, 0), (1, 1), (3, 0), (3, 1),
                           (1, 2), (1, 3), (3, 2), (3, 3)])}
                run_round(0, 1, w1, [(0, 0, 0.105)])

            # projections done: xtr/wr freed
            # R2: pair 1, i-half 0
            run_round(1, 0, {}, [(0, 1, 0.148)])
            # R3: pair 1, i-half 1
            run_round(1, 1, {}, [(1, 0, 0.192)])

            # ---- tail: normalize + project i-half 0 (fills the last
            # exchange's rank-skew window), then read + finish i-half 1
            for p in range(2):
                emit_norm(p, 0, 0.195)
            for p in range(2):
                for g in range(4):
                    emit_bcmul(p, 0, g, 0.196)
            for io in range(2):
                for nh in range(2):
                    outproj_group(0, io, nh, 0.198)
            emit_chunk_reads(1, 1, 0.202)
            for p in range(2):
                emit_norm(p, 1, 0.205)
            for p in range(2):
                for g in range(4):
                    emit_bcmul(p, 1, g, 0.206)
            for io in range(2):
                for nh in range(2):
                    outproj_group(1, io, nh, 0.208)

    nc.compile()
    return nc


def _get_nc():
    global _cached_nc
    if _cached_nc is None:
        _cached_nc = _build()
    return _cached_nc


def kernel(x, mask, W_qkv, W_out, b_out):
    x = np.asarray(x, dtype=np.float32)
    mask = np.asarray(mask)
    W_qkv = np.asarray(W_qkv, dtype=np.float32)
    W_out = np.asarray(W_out, dtype=np.float32)
    b_out = np.asarray(b_out, dtype=np.float32)

    nc = _get_nc()

    FP8 = ml_dtypes.float8_e4m3
    maskt_fp8 = np.ascontiguousarray(mask.reshape(N, N).T).astype(FP8)
    wout_bf = W_out.astype(BF16)
    # normalization selector: e8[s, 128g + r] = 1 iff s == 2g + r//64
    # (gathered sums live at partition 2g + hh)
    e8 = np.zeros((8, 512), dtype=np.float32)
    for g in range(4):
        for r in range(128):
            e8[2 * g + r // 64, 128 * g + r] = 1.0
    e8 = np.ascontiguousarray(e8).astype(BF16)

    in_maps = []
    for c in range(N_CORES):
        b = c // 4
        g = c % 4
        hs = slice(g * HPC * HD, (g + 1) * HPC * HD)  # 256 cols of this core
        wq = W_qkv[:, 0 * D:1 * D][:, hs] * np.float32(SCALE)
        wk = W_qkv[:, 1 * D:2 * D][:, hs]
        wv = W_qkv[:, 2 * D:3 * D][:, hs]
        wqkv_c = np.ascontiguousarray(
            np.concatenate([wq, wk, wv], axis=1)).astype(BF16)
        xt_c = np.ascontiguousarray(x[b].T).astype(BF16)
        in_maps.append({
            "xt": xt_c,
            "wqkv": wqkv_c,
            "maskt": maskt_fp8,
            "wout": wout_bf,
            "e8": e8,
        })

    global _last_in_maps, _last_res
    _last_in_maps = in_maps

    res = bass_utils.run_bass_kernel_spmd(
        nc, in_maps, core_ids=list(range(N_CORES)))
    _last_res = res

    out_full = np.empty((B, N, D), dtype=np.float32)
    for c in range(N_CORES):
        b = c // 4
        g = c % 4
        core_out = res.results[c]["out"].astype(np.float32)
        out_full[b, 256 * g:256 * g + 256, :] = core_out[0:256]
        out_full[b, 1024 + 256 * g:1024 + 256 * g + 256, :] = core_out[256:512]
    out_full += b_out
    return out_full
